# revision 18
# baseline (speedup 1.0000x reference)
"""Trainium2 Bass kernel for nn_Cond_PlanarTrans (conditional planar flow, MoE-routing).

Math (per batch b, particle i):
    w = relu(o @ W1.T + b1).reshape(B, 8, 64)
    u = relu(o @ W2.T + b2).reshape(B, 8, 64)
    bf = relu(o @ W3.T + b3).reshape(B, 8)
    n = m[b, i]
    pre = <s_t[b,i,:], w[b,n,:]> + bf[b,n]
    out[b,i,:] = s_t[b,i,:] + u[b,n,:] * tanh(pre)

Strategy: data-parallel over B across 8 cores (16 batches each). On each core:
  - tiny MLP computed once on the PE (weights transposed on-chip via PE transpose)
  - the per-particle mixture gather is a one-hot matmul on the PE:
        G[p, :] = onehotT.T @ [w | bf | u]  ->  [128, 129] per 128-particle chunk
    (one-hot masks are built host-side from m, shipped as an f32 input)
  - dot product + bias via a single elementwise mul (ones-column trick) + free-dim
    reduce on DVE; tanh on ACT; update u_m*t + s_t split between ACT (per-partition
    scale) + DVE adds and fused scalar_tensor_tensor on DVE.

Particle layout: partition p of a batch holds particles 16p..16p+15 (contiguous 4KB
per partition -> full-rate DMA); chunk j of a batch = particles {16p+j}.
"""

import os
import sys

import numpy as np

B, P, DIM, N_M = 128, 2048, 64, 8
NCORES = 8
BL = B // NCORES  # batches per core
JC = 16           # chunks per batch (particle = 16*p + j)
G = 8             # chunks per PSUM group (2 groups per batch)
WUC = 2 * DIM + 1  # 129 columns: [w (64) | bf (1) | u (64)]

# tunables
XB = int(os.environ.get("PK_XB", "8"))   # chunks per group on the grouped DVE broadcast-mul path
NT = int(os.environ.get("PK_NT", "4"))   # s_t tile ring depth
GP_ADD = bool(int(os.environ.get("PK_GP_ADD", "1")))  # final add on gpsimd (else DVE)
BCAST = bool(int(os.environ.get("PK_BCAST", "1")))    # grouped broadcast mul (else per-chunk stt)
RDV = int(os.environ.get("PK_RDV", "4"))  # chunks per group reduced on DVE (rest: ACT accum)

LAST_EXEC_NS = None
LAST_RESULTS = None

_CACHE = {}


def _import_concourse():
    try:
        import concourse.bass  # noqa: F401
    except ImportError:
        for p in ("/opt/trn_rl_repo", "/root/.axon_site/_ro/trn_rl_repo"):
            if os.path.isdir(p) and p not in sys.path:
                sys.path.insert(0, p)
        import concourse.bass  # noqa: F401


def _ensure_ntff_hook():
    """Provide antenv.axon_hooks (get/set_axon_ntff_profile_hook) if the image
    lacks it, wiring the NTFF profile capture directly to libaxon_pjrt.so."""
    try:
        from antenv.axon_hooks import get_axon_ntff_profile_hook  # noqa: F401
        return
    except ImportError:
        pass

    import contextlib
    import ctypes
    import types

    so_path = os.environ.get("AXON_PJRT_SO", "/opt/axon/libaxon_pjrt.so")
    hook = None
    if os.path.exists(so_path):
        lib = ctypes.CDLL(so_path)
        if hasattr(lib, "axon_start_nrt_profile"):
            lib.axon_start_nrt_profile.argtypes = [
                ctypes.POINTER(ctypes.c_int64),
                ctypes.c_size_t,
            ]
            lib.axon_start_nrt_profile.restype = ctypes.c_int64
            lib.axon_stop_nrt_profile.argtypes = [ctypes.c_char_p]
            lib.axon_stop_nrt_profile.restype = ctypes.c_int64

            @contextlib.contextmanager
            def hook(output_dir, device_ids):  # noqa: F811
                import jax

                jax.devices()
                if device_ids:
                    ids = (ctypes.c_int64 * len(device_ids))(*device_ids)
                    rc = lib.axon_start_nrt_profile(ids, len(device_ids))
                else:
                    rc = lib.axon_start_nrt_profile(None, 0)
                if rc != 0:
                    raise RuntimeError(f"axon_start_nrt_profile rc={rc}")
                try:
                    yield
                finally:
                    n = lib.axon_stop_nrt_profile(str(output_dir).encode())
                    print(f"profile: {n} file(s) written to {output_dir}")

    state = {"hook": hook}
    mod = types.ModuleType("antenv.axon_hooks")
    mod.get_axon_ntff_profile_hook = lambda: state["hook"]

    def _set(h):
        state["hook"] = h

    mod.set_axon_ntff_profile_hook = _set
    import antenv

    antenv.axon_hooks = mod
    sys.modules["antenv.axon_hooks"] = mod


def _build_bass():
    _import_concourse()
    from contextlib import ExitStack

    import concourse.bacc as bacc
    import concourse.bass as bass  # noqa: F401
    import concourse.tile as tile
    from concourse import mybir
    from concourse.masks import make_identity

    f32 = mybir.dt.float32
    bf16 = mybir.dt.bfloat16  # noqa: F841
    f16 = mybir.dt.float16
    AF = mybir.ActivationFunctionType
    OP = mybir.AluOpType
    AX = mybir.AxisListType

    # Bacc (not plain Bass): its finalize() splits multi-sem waits into event
    # semaphores — TRN2 instructions carry at most one wait, and walrus rejects
    # the raw Tile output otherwise.
    nc = bacc.Bacc(None)

    s_t = nc.declare_dram_parameter("s_t", [BL, P, DIM], f32, isOutput=False)
    oh = nc.declare_dram_parameter("oh", [BL, N_M, P], f16, isOutput=False)
    o_in = nc.declare_dram_parameter("o", [BL, DIM], f32, isOutput=False)
    W1 = nc.declare_dram_parameter("W1", [N_M * DIM, DIM], f32, isOutput=False)
    b1 = nc.declare_dram_parameter("b1", [N_M * DIM], f32, isOutput=False)
    W2 = nc.declare_dram_parameter("W2", [N_M * DIM, DIM], f32, isOutput=False)
    b2 = nc.declare_dram_parameter("b2", [N_M * DIM], f32, isOutput=False)
    W3 = nc.declare_dram_parameter("W3", [N_M, DIM], f32, isOutput=False)
    b3 = nc.declare_dram_parameter("b3", [N_M], f32, isOutput=False)
    out = nc.declare_dram_parameter("out", [BL, P, DIM], f32, isOutput=True)

    with tile.TileContext(nc) as tc, ExitStack() as ctx:
        consts = ctx.enter_context(tc.tile_pool(name="consts", bufs=1))

        # ---------- phase 0: constants + per-batch MLP ----------
        ident = consts.tile([128, 128], f32)
        make_identity(nc, ident)
        ones_row = consts.tile([1, 128], f32)
        nc.vector.memset(ones_row, 1.0)

        w1_sb = consts.tile([128, 4, DIM], f32)
        nc.sync.dma_start(out=w1_sb, in_=W1[:].rearrange("(q r) k -> r q k", r=128))
        w2_sb = consts.tile([128, 4, DIM], f32)
        nc.sync.dma_start(out=w2_sb, in_=W2[:].rearrange("(q r) k -> r q k", r=128))
        w3_sb = consts.tile([N_M, DIM], f32)
        nc.sync.dma_start(out=w3_sb, in_=W3[:])
        b1_sb = consts.tile([1, N_M * DIM], f32)
        nc.sync.dma_start(out=b1_sb, in_=b1[:].rearrange("(a n) -> a n", a=1))
        b2_sb = consts.tile([1, N_M * DIM], f32)
        nc.sync.dma_start(out=b2_sb, in_=b2[:].rearrange("(a n) -> a n", a=1))
        b3_sb = consts.tile([1, N_M], f32)
        nc.sync.dma_start(out=b3_sb, in_=b3[:].rearrange("(a n) -> a n", a=1))
        o_sb = consts.tile([BL, DIM], f32)
        nc.sync.dma_start(out=o_sb, in_=o_in[:])

        with tc.tile_pool(name="mlp_ps", bufs=2, space="PSUM") as mlp_ps:
            # transposes: oT [64, BL]; W1T/W2T [64, 512]; W3T [64, 8]
            oT = consts.tile([DIM, BL], f32)
            pt_o = mlp_ps.tile([DIM, BL], f32, tag="pt")
            nc.tensor.transpose(pt_o, o_sb, ident[0:BL, 0:BL])
            nc.vector.tensor_copy(oT, pt_o)

            w1T = consts.tile([DIM, N_M * DIM], f32)
            w2T = consts.tile([DIM, N_M * DIM], f32)
            for src, dst in ((w1_sb, w1T), (w2_sb, w2T)):
                for q in range(4):
                    pt = mlp_ps.tile([DIM, 128], f32, tag="pt")
                    nc.tensor.transpose(pt, src[:, q, :], ident)
                    nc.vector.tensor_copy(dst[:, q * 128:(q + 1) * 128], pt)
            w3T = consts.tile([DIM, N_M], f32)
            pt_3 = mlp_ps.tile([DIM, N_M], f32, tag="pt")
            nc.tensor.transpose(pt_3, w3_sb, ident[0:N_M, 0:N_M])
            nc.vector.tensor_copy(w3T, pt_3)

            # MLP: x_all = relu(o @ W.T + b), bias preloaded via ones-matmul accumulate
            w_all = consts.tile([BL, N_M * DIM], f32)
            u_all = consts.tile([BL, N_M * DIM], f32)
            bf_all = consts.tile([BL, N_M], f32)
            for bsb, wT, dst in (
                (b1_sb, w1T, w_all),
                (b2_sb, w2T, u_all),
                (b3_sb, w3T, bf_all),
            ):
                n_cols = dst.shape[-1]
                ps = mlp_ps.tile([BL, n_cols], f32, tag="mlp")
                nc.tensor.matmul(ps, lhsT=ones_row[0:1, 0:BL], rhs=bsb,
                                 start=True, stop=False)
                nc.tensor.matmul(ps, lhsT=oT, rhs=wT, start=False, stop=True)
                nc.scalar.activation(out=dst, in_=ps, func=AF.Relu)

        # WU[n, b, :] = [w[b,n,:] | bf[b,n] | u[b,n,:]] — partition-reshape via a
        # DRAM bounce (6 DMAs total; per-batch SBUF-SBUF scatter would put ~48
        # DMA waits on the first consuming matmul, which walrus rejects)
        w_dram = nc.dram_tensor("w_scratch", [BL, N_M * DIM], f32)
        u_dram = nc.dram_tensor("u_scratch", [BL, N_M * DIM], f32)
        bf_dram = nc.dram_tensor("bf_scratch", [BL, N_M], f32)
        nc.sync.dma_start(out=w_dram[:], in_=w_all)
        nc.sync.dma_start(out=u_dram[:], in_=u_all)
        nc.sync.dma_start(out=bf_dram[:], in_=bf_all)
        # bf16 for the gather matmul (fp32 matmuls run 2 passes at 4x cost);
        # SWDGE dma does the fp32->bf16 cast inline
        wu = consts.tile([N_M, BL, WUC], f16)
        nc.gpsimd.dma_start(
            out=wu[:, :, 0:DIM],
            in_=w_dram[:].rearrange("b (n k) -> n b k", n=N_M),
        )
        nc.gpsimd.dma_start(
            out=wu[:, :, DIM:DIM + 1],
            in_=bf_dram[:].rearrange("b n -> n b"),
        )
        nc.gpsimd.dma_start(
            out=wu[:, :, DIM + 1:WUC],
            in_=u_dram[:].rearrange("b (n k) -> n b k", n=N_M),
        )

        # ---------- phase 1: main loop ----------
        # s_t tiles: contiguous [128, 16, 64] (4KB/partition DMA runs)
        tts = []
        for i in range(NT):
            tt = consts.tile([128, JC, DIM], f32, tag=f"tt{i}", name=f"tt{i}")
            tts.append(tt)
        # write-only scratch for ACT-accumulate reduces
        junk = consts.tile([128, G, DIM], f32, name="junk")

        ohpool = ctx.enter_context(tc.tile_pool(name="ohpool", bufs=3))
        outpool = ctx.enter_context(tc.tile_pool(name="outpool", bufs=4))
        prpool = ctx.enter_context(tc.tile_pool(name="prpool", bufs=3))
        smpool = ctx.enter_context(tc.tile_pool(name="smpool", bufs=8))
        updpool = ctx.enter_context(tc.tile_pool(name="updpool", bufs=4))
        pspool = ctx.enter_context(tc.tile_pool(name="pspool", bufs=2, space="PSUM"))

        for b in range(BL):
            ttile = tts[b % NT]
            nc.sync.dma_start(
                out=ttile,
                in_=s_t[b].rearrange("(p j) k -> p j k", j=JC),
            )
            ohs = ohpool.tile([N_M, P], f16, tag="ohs")
            nc.sync.dma_start(out=ohs, in_=oh[b])
            outt = outpool.tile([128, JC, DIM], f32, tag="outt")

            # --- software-pipelined per-batch body: compute tanh inputs for
            # both half-batches first, then the update sweep, so the ACT tanh
            # round-trip doesn't stall the DVE stream ---
            pss, ths, upds = [], [], []
            for g in range(2):
                ps = pspool.tile([128, G, 256], f32, tag="ps")
                pss.append(ps)
                for jj in range(G):
                    j = g * G + jj
                    nc.tensor.matmul(
                        ps[:, jj, 0:WUC],
                        lhsT=ohs[:, j * 128:(j + 1) * 128],
                        rhs=wu[:, b, :],
                        start=True, stop=True,
                    )
                tsl = ttile[:, g * G:(g + 1) * G, :]
                prod = prpool.tile([128, G, DIM], f32, tag="prod")
                nc.vector.tensor_tensor(
                    out=prod, in0=tsl, in1=ps[:, :, 0:DIM], op=OP.mult,
                )
                pre = smpool.tile([128, G], f32, tag="pre")
                # reduce split: DVE takes the first RDV chunks, ACT the rest
                # (via activation accumulate) — balances the two queues
                if RDV < G:
                    nc.vector.reduce_sum(
                        out=pre[:, 0:RDV], in_=prod[:, 0:RDV, :], axis=AX.X,
                    )
                    for jj in range(RDV, G):
                        nc.scalar.activation(
                            out=junk[:, jj, :], in_=prod[:, jj, :],
                            func=AF.Copy, bias=0.0,
                            accum_out=pre[:, jj:jj + 1],
                        )
                else:
                    nc.vector.reduce_sum(out=pre, in_=prod, axis=AX.X)
                # + b_m (column 64 of each chunk's PSUM slice)
                pre2 = smpool.tile([128, G], f32, tag="pre2")
                nc.vector.tensor_tensor(
                    out=pre2, in0=pre, in1=ps[:, :, DIM], op=OP.add,
                )
                th = smpool.tile([128, G], f32, tag="th")
                nc.scalar.activation(out=th, in_=pre2, func=AF.Tanh)
                ths.append(th)

            for g in range(2):
                ps, th = pss[g], ths[g]
                tsl = ttile[:, g * G:(g + 1) * G, :]
                # upd[:, jj, :] = u_m * t — grouped broadcast mul on DVE
                upd = updpool.tile([128, G, DIM], f32, tag="upd")
                nxb = XB if BCAST else 0
                if nxb:
                    th_b = bass.AP(
                        tensor=th.tensor,
                        offset=th.offset,
                        ap=[th.ap[0], [th.ap[1][0], nxb], [0, DIM]],
                    )
                    nc.vector.tensor_tensor(
                        out=upd[:, 0:nxb, :],
                        in0=ps[:, 0:nxb, DIM + 1:WUC],
                        in1=th_b,
                        op=OP.mult,
                    )
                for jj in range(nxb, G):
                    nc.scalar.activation(
                        out=upd[:, jj, :],
                        in_=ps[:, jj, DIM + 1:WUC],
                        func=AF.Copy,
                        bias=0.0,
                        scale=th[:, jj:jj + 1],
                    )
                add_eng = nc.gpsimd if GP_ADD else nc.vector
                add_eng.tensor_tensor(
                    out=outt[:, g * G:(g + 1) * G, :],
                    in0=upd, in1=tsl, op=OP.add,
                )

            nc.sync.dma_start(
                out=out[b].rearrange("(p j) k -> p j k", j=JC),
                in_=outt,
            )

    # Bacc defers register allocation to finalize(); run_bass_via_pjrt
    # serializes nc as-is, so finalize here.
    nc.finalize()
    return nc


def _get_bass():
    if "nc" not in _CACHE:
        _CACHE["nc"] = _build_bass()
    return _CACHE["nc"]


def kernel(m, s_t, o, W1, b1, W2, b2, W3, b3):
    global LAST_EXEC_NS, LAST_RESULTS
    _import_concourse()
    from concourse.bass_utils import run_bass_kernel_spmd

    m = np.asarray(m)
    s_t = np.ascontiguousarray(np.asarray(s_t, dtype=np.float32))
    o = np.ascontiguousarray(np.asarray(o, dtype=np.float32))
    W1 = np.ascontiguousarray(np.asarray(W1, dtype=np.float32))
    b1 = np.ascontiguousarray(np.asarray(b1, dtype=np.float32))
    W2 = np.ascontiguousarray(np.asarray(W2, dtype=np.float32))
    b2 = np.ascontiguousarray(np.asarray(b2, dtype=np.float32))
    W3 = np.ascontiguousarray(np.asarray(W3, dtype=np.float32))
    b3 = np.ascontiguousarray(np.asarray(b3, dtype=np.float32))

    # one-hot masks, laid out [B, n, j*128 + p] with particle = 16*p + j
    # (fp16: 0.0/1.0 exact; 2-byte matmuls run ~4x faster than fp32, and fp16
    # keeps w/u/bf rounding ~8x tighter than bf16)
    import ml_dtypes

    mr = m.reshape(B, 128, JC).transpose(0, 2, 1)  # [B, j, p]
    ohf = (mr[:, None, :, :] == np.arange(N_M)[None, :, None, None])
    ohf = np.ascontiguousarray(ohf.reshape(B, N_M, P).astype(np.float16))

    nc = _get_bass()
    in_maps = []
    for c in range(NCORES):
        sl = slice(c * BL, (c + 1) * BL)
        in_maps.append({
            "s_t": s_t[sl], "oh": ohf[sl], "o": o[sl],
            "W1": W1, "b1": b1, "W2": W2, "b2": b2, "W3": W3, "b3": b3,
        })

    trace = bool(os.environ.get("BASS_KERNEL_TRACE"))
    if trace:
        _ensure_ntff_hook()
    res = run_bass_kernel_spmd(nc, in_maps, list(range(NCORES)), trace=trace)
    LAST_EXEC_NS = res.exec_time_ns
    LAST_RESULTS = res

    outp = np.concatenate([res.results[i]["out"] for i in range(NCORES)], axis=0)
    return outp.reshape(B, P, DIM).astype(np.float32, copy=False)
